# revision 37
# baseline (speedup 1.0000x reference)
"""BPNet GNN message-passing kernel for 8 Trainium2 NeuronCores.

Strategy (no indirect DMA / no extended-GPSIMD ucode on this image; only
static DMA + PE + DVE/ACT compute is usable):
  - Node-sharded output: the host assigns the 4096 nodes to 32 chunks of
    128 (core c owns 4 chunks = the rows of its [128, 52] output). The
    assignment is load-balanced (greedy vector bin-packing on per-slot
    node degrees + swap local search) until every (core, chunk, slot)
    pair-group is exactly <= 512 -> 48 tiles per core, zero padding.
  - Every (edge,slot) pair is routed to the owner core of its target
    node and baked host-side into dense device tensors:
      xab  [128, L]  fp8  : one-hot-placed gathered node features + type
                            indicator for the pair's two COMPANION slots
                            (K-dim one-hot folds per-edge weight choice
                            into one fixed stage-1 matmul).
      oneh [128,SB,4,128] fp8 : per-tile one-hot incidence lane -> row.
      msl  [128,SB,4,4,13] bf16 : per-pair output-type select mask.
      biast [1,MB,2,4,4,13] bf16 : ho_bias rows, added into stage-2 PSUM
                            via a K=1 ones-matmul BEFORE selection (the
                            mask then keeps only the pair's own block).
  - Device, 3-stage software pipeline over megablocks (2 superblocks = 8
    tiles) so PE never waits on the ACT->DVE select chain:
      stage1(m):  pa/pb = w1{a,b}^T @ xab (PSUM); tb = relu(pb) [ACT];
                  fact = max(pa,0)*tb [one fused DVE scalar_tensor_tensor]
      stage2(m-1): per tile psum[128,4,13] = fact_t^T @ w2[slot] (+bias
                  ones-matmul); pc = copy bf16 [ACT]; sel = pc*msl,
                  f1/f2 = 4->1 block-sum [DVE, batched per megablock]
      seg(m-2):   psum_nodes[:, 13q:+13] += oneh_t^T @ f2_t; each q
                  chunk is drained to HBM as soon as its group closes.
  - DMA: every dma_start costs ~0.7us on its issuing engine and ~0.9us
    semaphore latency, so few large chunks are issued from SP (weights +
    xab), ACT (2nd xab chunk) and GPSIMD (oneh/msl) in first-need order.
  - ~8 dummy matmuls during the DMA lead-in keep the PE continuously
    busy so real matmuls start in the full-speed pstate (2x).
  - Output per core: [128, 52] f32; unshard on host via the node map.
  - Accuracy: fp8(e4m3) for the gathered features costs ~5.5e-3 relative
    error (budget 2e-2); all accumulation stays f32/bf16.
"""

import numpy as np
import ml_dtypes

N, E, ORDER, D, RANK = 4096, 16384, 3, 13, 128
NP_ = ORDER + 1  # 4 types
NCORES = 8
NODES_PER_CORE = N // NCORES  # 512
NCHUNKS = 32  # 32 chunks of 128 nodes

bf16 = ml_dtypes.bfloat16
fp8 = ml_dtypes.float8_e4m3

_COMPILED = {}  # tuple(Tg) -> nc


def _tile_map(Tgs):
    """tile t -> (q, i); per-q first/last tile index."""
    tq, ti = [], []
    for g, tg in enumerate(Tgs):
        q, i = g // 3, g % 3
        tq += [q] * tg
        ti += [i] * tg
    qstart = {}
    qend = {}
    for t, q in enumerate(tq):
        if q not in qstart:
            qstart[q] = t
        qend[q] = t
    return tq, ti, qstart, qend


def _build_program(Tgs):
    import concourse.bacc as bacc
    import concourse.tile as tile
    from concourse import mybir

    T = sum(Tgs)
    SB = T // 4
    L = 128 * T
    tq, ti, qstart, qend = _tile_map(Tgs)

    nc = bacc.Bacc("TRN2", target_bir_lowering=False, debug=False,
                   num_devices=NCORES)
    BF, F32 = mybir.dt.bfloat16, mybir.dt.float32
    F8 = mybir.dt.float8e4
    Relu = mybir.ActivationFunctionType.Relu
    Copy = mybir.ActivationFunctionType.Copy
    mult = mybir.AluOpType.mult
    amax = mybir.AluOpType.max
    aadd = mybir.AluOpType.add
    abyp = mybir.AluOpType.bypass

    xab = nc.dram_tensor("xab", [128, L], F8, kind="ExternalInput").ap()
    w1a = nc.dram_tensor("w1a", [128, 128], BF, kind="ExternalInput").ap()
    w1b = nc.dram_tensor("w1b", [128, 128], BF, kind="ExternalInput").ap()
    w2 = nc.dram_tensor("w2", [128, 3, 52], BF, kind="ExternalInput").ap()
    # per-tile one-hot incidence (lane -> node row), exact 0/1 -> fp8
    oneh = nc.dram_tensor("oneh", [128, SB, 4, 128], F8,
                          kind="ExternalInput").ap()
    # per-pair type-select mask over the 4 packed 13-blocks (bf16: fp8
    # would disable the DVE 2x/4x fast modes)
    msl = nc.dram_tensor("msl", [128, SB, 4, 4, 13], BF,
                         kind="ExternalInput").ap()
    # ho_bias rows per (tile, type-block); added into stage-2 PSUM via a
    # K=1 ones-matmul BEFORE selection (the mask then picks the right one)
    biast = nc.dram_tensor("biast", [1, SB // 2, 2, 4, 4, 13], BF,
                           kind="ExternalInput").ap()
    out = nc.dram_tensor("out", [128, 52], F32, kind="ExternalOutput").ap()

    # DMA chunk boundaries (in superblocks). xab chunks go on SP, oneh/
    # mask chunks on GPSIMD, weights on ACT -- each engine's dma_start
    # issues serialize at ~700ns apiece, so spreading them is critical.
    xbounds = [0, 1, 5, 9, SB]
    xchunks = [(xbounds[j], xbounds[j + 1]) for j in range(len(xbounds) - 1)
               if xbounds[j] < xbounds[j + 1]]
    sbounds = list(range(0, SB, 4)) + [SB]
    schunks = [(sbounds[j], sbounds[j + 1]) for j in range(len(sbounds) - 1)
               if sbounds[j] < sbounds[j + 1]]
    MB = SB // 2  # megablock = 2 superblocks = 8 tiles

    with tile.TileContext(nc) as tc:
        with tc.tile_pool(name="inp", bufs=1) as inp, \
             tc.tile_pool(name="work", bufs=1) as work, \
             tc.tile_pool(name="tbp", bufs=4) as tbp, \
             tc.tile_pool(name="factp", bufs=4) as factp, \
             tc.tile_pool(name="selp", bufs=2) as selp, \
             tc.tile_pool(name="pcp", bufs=2) as pcp, \
             tc.tile_pool(name="f1p", bufs=2) as f1p, \
             tc.tile_pool(name="f2p", bufs=3) as f2p, \
             tc.tile_pool(name="ps1", bufs=6, space="PSUM") as ps1, \
             tc.tile_pool(name="ps2", bufs=1, space="PSUM") as ps2, \
             tc.tile_pool(name="psn", bufs=1, space="PSUM") as psn:

            w1a_sb = inp.tile([128, 128], BF, tag="w1a")
            w1b_sb = inp.tile([128, 128], BF, tag="w1b")
            w2_sb = inp.tile([128, 3, 52], BF, tag="w2")
            xab_chs = {}
            strm_chs = {}
            msl_chs = {}
            for (lo, hi) in xchunks:
                xab_chs[lo] = inp.tile([128, (hi - lo) * 512], F8,
                                       tag=f"xabc{lo}", name=f"xabc{lo}")
            for (lo, hi) in schunks:
                strm_chs[lo] = inp.tile([128, hi - lo, 4, 128], F8,
                                        tag=f"strmc{lo}", name=f"strmc{lo}")
                msl_chs[lo] = inp.tile([128, hi - lo, 4, 4, 13], BF,
                                       tag=f"mslc{lo}", name=f"mslc{lo}")

            def xab_sb(s):  # [128, 512] view of superblock s
                for (lo, hi) in xchunks:
                    if lo <= s < hi:
                        return xab_chs[lo][:, 512 * (s - lo):512 * (s - lo + 1)]

            def oneh_tile(s, k):  # [128, 128] fp8 one-hot for tile 4s+k
                for (lo, hi) in schunks:
                    if lo <= s < hi:
                        return strm_chs[lo][:, s - lo, k]

            def msl_mb(m):  # [128, 2, 4, 5, 13] mask+bias for megablock m
                for (lo, hi) in schunks:
                    if lo <= 2 * m < hi:
                        return msl_chs[lo][:, 2 * m - lo:2 * m - lo + 2]

            warm = work.tile([128, 1], F32, tag="warm")
            oc = work.tile([128, 52], F32, tag="oc")
            scratch = work.tile([128, 512], BF, tag="scratch")
            biast_sb = work.tile([1, MB, 2, 4, 4, 13], BF, tag="biast")
            ones1 = work.tile([1, 128], BF, tag="ones1")

            # DVE: constants, then compute.
            nc.vector.memset(ones1[:], 1.0)
            nc.vector.memset(warm[:], 0.0)
            # ACT: warm up the Relu table immediately, then issue the
            # second xab chunk in parallel with SP's issue stream.
            nc.scalar.activation(warm[:], warm[:], Relu)
            (lo1, hi1) = xchunks[1]
            nc.scalar.dma_start(xab_chs[lo1][:], xab[:, 512 * lo1:512 * hi1])
            # SP: weights + xab in strict first-need order; few, large
            # chunks amortize the ~0.7us issue + ~0.9us semaphore latency.
            (lo0, hi0) = xchunks[0]
            nc.sync.dma_start(w1a_sb[:], w1a[:])
            nc.sync.dma_start(xab_chs[lo0][:], xab[:, 512 * lo0:512 * hi0])
            nc.sync.dma_start(w1b_sb[:], w1b[:])
            nc.sync.dma_start(w2_sb[:, :, :], w2[:, :, :])
            nc.sync.dma_start(biast_sb[:, :, :, :, :, :],
                              biast[:, :, :, :, :, :])
            for (lo, hi) in xchunks[2:]:
                nc.sync.dma_start(xab_chs[lo][:], xab[:, 512 * lo:512 * hi])
            # GPSIMD: oneh + mask chunks (first needed only at stage2/seg,
            # which runs well after GPSIMD's late start).
            for (lo, hi) in schunks:
                nc.gpsimd.dma_start(msl_chs[lo][:, :, :, :, :],
                                    msl[:, lo:hi, :, :, :])
                nc.gpsimd.dma_start(strm_chs[lo][:, :, :, :],
                                    oneh[:, lo:hi, :, :])

            # Pre-ramp the PE during the DMA lead-in: dummy matmuls keep
            # it continuously busy so the real stage-1 matmuls start at the
            # full-speed pstate (2x the mid-pstate rate).
            nc.vector.memset(scratch[:], 0.0)
            for w_ in range(8):
                pd = ps1.tile([128, 512], F32, tag="p1", name=f"pd{w_}")
                nc.tensor.matmul(pd[:], scratch[:, 0:128], scratch[:],
                                 start=True, stop=True)

            pn = psn.tile([128, 52], F32, tag="pn")

            def stage1(s, mode):
                pa = ps1.tile([128, 512], F32, tag="p1", name=f"pa{s}")
                pb = ps1.tile([128, 512], F32, tag="p1", name=f"pb{s}")
                nc.tensor.matmul(pa[:], w1a_sb[:], xab_sb(s),
                                 start=True, stop=True)
                nc.tensor.matmul(pb[:], w1b_sb[:], xab_sb(s),
                                 start=True, stop=True)
                fact = factp.tile([128, 512], BF, tag="fact", name=f"fact{s}")
                tb = tbp.tile([128, 512], BF, tag="tb", name=f"tb{s}")
                nc.scalar.activation(tb[:], pb[:], Relu)
                nc.vector.scalar_tensor_tensor(
                    fact[:], pa[:], 0.0, tb[:], amax, mult)
                return fact

            def stage2(m, facts):
                # megablock m = superblocks 2m, 2m+1 = tiles 8m .. 8m+7
                pb4 = ps2.tile([128, 2, 4, 4, 13], F32, tag="p2",
                               name=f"pb4_{m}")
                for j in range(2):
                    for k in range(4):
                        t = 8 * m + 4 * j + k
                        nc.tensor.matmul(
                            pb4[:, j, k, :, :],
                            facts[j][:, 128 * k:128 * (k + 1)],
                            w2_sb[:, ti[t], :],
                            start=(j == 0 and k == 0), stop=False,
                            skip_group_check=True)
                nc.tensor.matmul(pb4[:, :, :, :, :], ones1[:],
                                 biast_sb[:, m, :, :, :, :], start=False,
                                 stop=True, skip_group_check=True)
                # PSUM -> SBUF bf16 copy on ACT, then the all-bf16 (4x DVE
                # rate) mask-select and 4->1 block sum on DVE, batched over
                # the whole megablock to amortize per-op overheads.
                pc = pcp.tile([128, 2, 4, 4, 13], BF, tag="pc", name=f"pc{m}")
                nc.scalar.activation(pc[:, :, :, :, :], pb4[:, :, :, :, :],
                                     Copy)
                sel = selp.tile([128, 2, 4, 4, 13], BF, tag="sel",
                                name=f"sel{m}")
                nc.vector.tensor_tensor(
                    sel[:, :, :, :, :], pc[:, :, :, :, :], msl_mb(m), mult)
                f1 = f1p.tile([128, 2, 4, 2, 13], BF, tag="f1", name=f"f1_{m}")
                f2 = f2p.tile([128, 2, 4, 13], BF, tag="f2", name=f"f2_{m}")
                nc.vector.tensor_tensor(
                    f1[:, :, :, :, :], sel[:, :, :, 0:2, :],
                    sel[:, :, :, 2:4, :], aadd)
                nc.vector.tensor_tensor(
                    f2[:, :, :, :], f1[:, :, :, 0, :], f1[:, :, :, 1, :],
                    aadd)
                return f2

            def seg(m, f2):
                for j in range(2):
                    for k in range(4):
                        t = 8 * m + 4 * j + k
                        q = tq[t]
                        nc.tensor.matmul(
                            pn[:, 13 * q:13 * (q + 1)],
                            oneh_tile(2 * m + j, k), f2[:, j, k, :],
                            start=(t == qstart[q]), stop=(t == qend[q]),
                            skip_group_check=True)
                        if t == qend[q]:
                            # q's accumulation just closed: drain it now so
                            # the final output DMA isn't on the tail.
                            nc.vector.tensor_copy(
                                oc[:, 13 * q:13 * (q + 1)],
                                pn[:, 13 * q:13 * (q + 1)])
                            nc.sync.dma_start(
                                out[:, 13 * q:13 * (q + 1)],
                                oc[:, 13 * q:13 * (q + 1)])

            # 3-stage pipeline: PE never waits on the ACT->DVE select
            # chain -- seg matmuls run one full megablock after stage2.
            prev_f = None
            prev_f2 = None
            for m in range(MB):
                f0 = stage1(2 * m, "A")
                f1_ = stage1(2 * m + 1, "A")
                if prev_f is not None:
                    nf2 = stage2(m - 1, prev_f)
                    if prev_f2 is not None:
                        seg(m - 2, prev_f2)
                    prev_f2 = nf2
                prev_f = (f0, f1_)
            prev_f2_last = stage2(MB - 1, prev_f)
            seg(MB - 2, prev_f2)
            seg(MB - 1, prev_f2_last)

    nc.compile()
    return nc


def _balance(edges):
    """Assign nodes to 32 chunks of <=128, balancing per-slot pair counts;
    then chunks -> (core, q) with the heaviest chunks concentrated in the
    same q so overflow tiles are shared. Returns (c_of, q_of, r_of, Tgs)."""
    deg = np.zeros((N, ORDER), np.int64)
    for i in range(ORDER):
        np.add.at(deg[:, i], edges[:, i], 1)
    order = np.argsort(-deg.sum(1), kind="stable")
    loads = np.zeros((NCHUNKS, ORDER), np.float64)
    counts = np.zeros(NCHUNKS, np.int64)
    chunk_of = np.empty(N, np.int64)
    for n in order:
        d = deg[n]
        cand = np.maximum(loads + d, 0).max(1) + 1e-3 * (loads + d).sum(1)
        cand[counts >= 128] = 1e18
        ch = int(np.argmin(cand))
        chunk_of[n] = ch
        loads[ch] += d
        counts[ch] += 1

    # Local search: per-slot totals are exactly 32*512, so the optimum is
    # every (chunk, slot) load == 512 (zero overflow -> 4 tiles per group).
    # Swap nodes between chunks while the swap reduces total overflow.
    def overflow(ld):
        return np.maximum(ld - 512.0, 0.0).sum()

    degf = deg.astype(np.float64)
    rng = np.random.default_rng(0)
    stall = 0
    for _ in range(20000):
        over_by_chunk = np.maximum(loads - 512.0, 0.0).sum(1)
        if over_by_chunk.sum() == 0:
            break
        A = int(np.argmax(over_by_chunk + 1e-6 * rng.random(NCHUNKS)))
        nodes_A = np.nonzero(chunk_of == A)[0]
        want = np.maximum(loads[A] - 512.0, 0.0)
        n_cand = nodes_A[np.argsort(-(degf[nodes_A] @ want))[:8]]
        fA0 = overflow(loads[A][None, :])
        best = (0.0, None, None)
        for n in n_cand:
            dn = degf[n]
            # candidate partners: every other node
            dm = degf  # [N, 3]
            newA = loads[A][None, :] - dn[None, :] + dm
            fB0 = np.maximum(loads[chunk_of] - 512.0, 0.0).sum(1)
            newB = loads[chunk_of] + dn[None, :] - dm
            delta = (np.maximum(newA - 512.0, 0.0).sum(1) - fA0
                     + np.maximum(newB - 512.0, 0.0).sum(1) - fB0)
            delta[chunk_of == A] = 1e18
            m = int(np.argmin(delta))
            if delta[m] < best[0] - 1e-9:
                best = (float(delta[m]), int(n), m)
        if best[1] is None:
            stall += 1
            if stall > 20:
                break
            continue
        stall = 0
        _, n, m = best
        B = chunk_of[m]
        loads[A] += degf[m] - degf[n]
        loads[B] += degf[n] - degf[m]
        chunk_of[n], chunk_of[m] = B, A

    # chunks ranked by worst slot load; rank j -> core j%8, q j//8
    rank = np.argsort(-loads.max(1), kind="stable")
    core_of_chunk = np.empty(NCHUNKS, np.int64)
    q_of_chunk = np.empty(NCHUNKS, np.int64)
    for j, ch in enumerate(rank):
        core_of_chunk[ch] = j % NCORES
        q_of_chunk[ch] = j // NCORES
    c_of = core_of_chunk[chunk_of]
    q_of = q_of_chunk[chunk_of]
    # r: position within chunk
    r_of = np.empty(N, np.int64)
    for ch in range(NCHUNKS):
        nodes = np.nonzero(chunk_of == ch)[0]
        r_of[nodes] = np.arange(len(nodes))
    return c_of, q_of, r_of


def _prep_inputs(nodes, bp_params, bp_bias, ho_params, ho_bias, edges,
                 edge_types):
    nodes = np.asarray(nodes, np.float32)
    bp_params = np.asarray(bp_params, np.float32)
    bp_bias = np.asarray(bp_bias, np.float32)
    ho_params = np.asarray(ho_params, np.float32)
    ho_bias = np.asarray(ho_bias, np.float32)
    edges = np.asarray(edges, np.int64)
    edge_types = np.asarray(edge_types, np.int64)

    nodes_b = nodes.astype(bf16)

    c_of, q_of, r_of = _balance(edges)

    # group sizes per (core, q, i)
    cnt = np.zeros((NCORES, 4, ORDER), np.int64)
    tgt_c = c_of[edges]   # [E, 3]
    tgt_q = q_of[edges]
    tgt_r = r_of[edges]
    for i in range(ORDER):
        np.add.at(cnt[:, :, i], (tgt_c[:, i], tgt_q[:, i]), 1)
    Tgs = [int(np.ceil(cnt[:, g // 3, g % 3].max() / 128)) for g in range(12)]
    pad = (-sum(Tgs)) % 8  # megablocks need T % 8 == 0
    Tgs[11] += pad
    Tgs = tuple(Tgs)
    T = sum(Tgs)
    SB = T // 4
    L = 128 * T
    off = np.concatenate([[0], np.cumsum(Tgs)]) * 128  # col offset per group

    # weight tables (shared across cores)
    w1a = np.zeros((128, 128), np.float32)
    w1b = np.zeros((128, 128), np.float32)
    for p in range(NP_):
        w1a[13 * p:13 * p + 13, :] = bp_params[p]
        w1a[52 + p, :] = bp_bias[p, 0, :]
        w1b[64 + 13 * p:64 + 13 * p + 13, :] = bp_params[p]
        w1b[116 + p, :] = bp_bias[p, 0, :]
    w2 = np.zeros((128, 3, 52), np.float32)
    for i in range(ORDER):
        for p in range(NP_):
            w2[:, i, 13 * p:13 * p + 13] = ho_params[i, p]
    w1a_b = w1a.astype(bf16)
    w1b_b = w1b.astype(bf16)
    w2_b = w2.astype(bf16)

    in_maps = []
    for c in range(NCORES):
        xab = np.zeros((128, L), np.float32)
        oneh = np.zeros((128, T, 128), np.float32)
        msl = np.zeros((128, T, 4, 13), np.float32)
        biast = np.zeros((1, T, 4, 13), np.float32)
        for qq in range(4):
            for i in range(ORDER):
                g = qq * 3 + i
                es = np.nonzero((tgt_c[:, i] == c) & (tgt_q[:, i] == qq))[0]
                k = np.arange(len(es))
                x = off[g] + k
                t_arr = x // 128
                lane = x % 128
                a, b = (i + 1) % 3, (i + 2) % 3
                ta_t = edge_types[es, a]
                tb_t = edge_types[es, b]
                fa = nodes_b[edges[es, a]].astype(np.float32)  # [m, 13]
                fb = nodes_b[edges[es, b]].astype(np.float32)
                for dd in range(D):
                    xab[13 * ta_t + dd, x] = fa[:, dd]
                    xab[64 + 13 * tb_t + dd, x] = fb[:, dd]
                xab[52 + ta_t, x] = 1.0
                xab[116 + tb_t, x] = 1.0
                p_e = edge_types[es, i]
                r_e = tgt_r[es, i]
                oneh[lane, t_arr, r_e] = 1.0
                msl[lane, t_arr, p_e, :] = 1.0
                for tt in np.unique(t_arr):
                    biast[0, tt, :, :] = ho_bias[i, :, 0, :]
        in_maps.append({
            "xab": xab.astype(fp8),
            "w1a": w1a_b, "w1b": w1b_b, "w2": w2_b,
            "oneh": oneh.reshape(128, SB, 4, 128).astype(fp8),
            "msl": msl.reshape(128, SB, 4, 4, 13).astype(bf16),
            "biast": biast.reshape(1, SB // 2, 2, 4, 4, 13).astype(bf16),
        })
    return in_maps, Tgs, (c_of, q_of, r_of)


def kernel(nodes, bp_params, bp_bias, ho_params, ho_bias, edges, edge_types,
           atoms=None, atom_edges=None, _run_kwargs=None):
    from concourse.bass_utils import run_bass_kernel_spmd

    in_maps, Tgs, (c_of, q_of, r_of) = _prep_inputs(
        nodes, bp_params, bp_bias, ho_params, ho_bias, edges, edge_types)
    if Tgs not in _COMPILED:
        _COMPILED[Tgs] = _build_program(Tgs)
    nc = _COMPILED[Tgs]

    res = run_bass_kernel_spmd(nc, in_maps, core_ids=list(range(NCORES)),
                               **(_run_kwargs or {}))
    outs = np.stack([res.results[c]["out"] for c in range(NCORES)])  # [8,128,52]
    full = outs[c_of, r_of][np.arange(N)[:, None],
                            13 * q_of[:, None] + np.arange(D)[None, :]]
    kernel._last_result = res
    return full.astype(np.float32)
